# revision 6
# baseline (speedup 1.0000x reference)
"""AtomicCharge Trainium2 kernel (nn_AtomicCharge_77781857730661).

Strategy
--------
Data-parallel over atoms across 8 NeuronCores. The host packs molecules
(contiguous runs of the sorted `batch` tensor) into 1024 partition-rows
(8 cores x 128 partitions) of capacity T slots (T=1984 when the data
packs that tight, else 2048), so every molecule lives contiguously
along the free dim of one partition. x is uploaded pre-transposed in
bf16 (halves HBM traffic vs fp32), in the j-major order the device
pipeline streams it. aux masks travel bf16; output returns bf16.

Per core (raw bass, explicit semaphores; waits are standalone):
  PE:  per group (4 chunks x NB atoms): 4x mm1 (W1^T x, bf16) col-tiled
       2x on the PE array -- chunks 0/2 via tile (0,0) into hp[0:64],
       chunks 1/3 via tile (0,64) into hp[64:128]; consecutive
       tile-0/tile-64 matmuls stream concurrently into one [128,2NB]
       PSUM tile per group. Then 2x K=128 mm2 with per-pair placement
       stationaries accumulate atom_out into the packed [128,NB] panel
       PSUM for block j (panels double-buffered; mm2 lagged 6 groups).
       The mm2 placement stationary w2p is BUILT ON DEVICE (gpsimd
       memset + 2 strided DVE broadcast-adds) instead of DMAing 2.1MB.
  ACT: ONE [128,2NB] Silu per group (bias b1 fused, bf16 out).
  DVE: per-j panel drains (+b2), chained per-block forward segmented
       scans and Q = (CH-FL)*IV + ao*(1+IV) precompute; the tail is a
       2-chunk reverse scan with the hi-half elementwise finish on
       GPSIMD in parallel, and the output DMAs per half:
       out = Q - rev(RL)*IV.

Pipelining: x slabs 4 groups each (~16KB/partition DMA lines), 4-deep;
hp PSUM triple-buffered; hs 8-deep.
The compile enables walrus redundant-LDWEIGHTS elimination.
HW: ~200 us/core on trn2 (8 cores); rel err ~4e-3 vs fp32 ref (bf16).
"""
import sys

sys.path.insert(0, "/opt/trn_rl_repo")

import numpy as np
import ml_dtypes

import concourse.bass as bass
from concourse import mybir
from concourse.bass_utils import run_bass_kernel_spmd
import concourse.bass_utils as _bu

# Enable walrus's redundant-LDWEIGHTS elimination (off by default in this
# stack); our mm1s reuse the same stationaries within a group.
if not getattr(_bu, "_ldwopt_patched", False):
    _orig_run_command = _bu.run_command

    def _run_command_ldwopt(argv, **kw):
        argv = [a.replace("--enable-ldw-opt=false", "--enable-ldw-opt=true")
                for a in argv]
        return _orig_run_command(argv, **kw)

    _bu.run_command = _run_command_ldwopt
    _bu._ldwopt_patched = True

F32 = mybir.dt.float32
BF16 = mybir.dt.bfloat16
NP_BF16 = ml_dtypes.bfloat16

# problem constants (hardcoded per spec)
N_ATOMS = 2_000_000
N_MOL = 50_000
D = 128      # node feature dim = SBUF partitions
H = 64       # hidden dim
NCORES = 8
R = 128      # atom-layout rows per core (partitions)
T_TIGHT = 1984
T_SAFE = 2048

_NC_CACHE = {}
LAST_RUN_INFO = {}


def build_raw(T, use_silu=True):
    """j-outer pipeline: groups of 2 pairs; col-tiled mm1 into one
    [128,2NB] PSUM tile; one [128,2NB] silu; K=128 mm2; panels
    double-buffered across j; incremental DVE epilogue."""
    NB = T // 4
    NPAIR = R // 2
    NBLK = 4
    NG = NPAIR // 2             # groups per block; group = 2 pairs = 4*NB atoms
    S = R * T
    CW = 512                    # chunk stride inside hp/hs (bank-aligned)
    WW = 2 * CW                 # hs width per group (pad cols beyond NB unused)
    GW = 4 * NB                 # xT columns per group (tight, no padding)
    XPG = 4                     # groups per x slab
    NXP = 4                     # x slab buffers
    NHS = 8                     # hs buffers
    MM2_LAG = 6                 # mm2 trails mm1 by this many groups
    T2 = T // 2
    AOp = mybir.AluOpType

    nc = bass.Bass()
    # xT is laid out j-major on the host: block j, then pair k, then
    # (row 2k | row 2k+1) x NB columns
    xT = nc.declare_dram_parameter("xT", [D, S], BF16, isOutput=False)
    W1 = nc.declare_dram_parameter("W1", [D, H], BF16, isOutput=False)
    b1s = nc.declare_dram_parameter("b1s", [D], F32, isOutput=False)
    b2 = nc.declare_dram_parameter("b2", [1], F32, isOutput=False)
    W2s = nc.declare_dram_parameter("W2s", [D, 1], F32, isOutput=False)
    aux = nc.declare_dram_parameter("aux", [R, 4 * T], BF16, isOutput=False)
    out = nc.declare_dram_parameter("out", [R, T], BF16, isOutput=True)

    from contextlib import ExitStack
    with ExitStack() as ctx:
        def sbuf(shape, dtype, name):
            return ctx.enter_context(nc.sbuf_tensor(name, shape, dtype))

        def psum(shape, name):
            return ctx.enter_context(nc.psum_tensor(name, shape, F32))

        w1a = sbuf([D, H], BF16, "w1a")
        w1b = sbuf([D, H], BF16, "w1b")
        b1t = sbuf([D, 1], F32, "b1t")
        b2t = sbuf([D, 1], F32, "b2t")
        w2s = sbuf([D, 1], F32, "w2s")
        w2p = sbuf([D, NPAIR * D], BF16, "w2p")
        auxt = sbuf([R, 4 * T], BF16, "auxt")
        xp = [sbuf([D, XPG * GW], BF16, f"xp{s}") for s in range(NXP)]
        hs = [sbuf([D, WW], BF16, f"hs{s}") for s in range(NHS)]
        ao = sbuf([R, T], F32, "ao")
        FL = sbuf([R, T], F32, "FL")
        RLr = sbuf([R, T], F32, "RLr")
        CH = sbuf([R, T], F32, "CH")
        IV = sbuf([R, T], F32, "IV")
        IVp = sbuf([R, T], F32, "IVp")
        obuf = sbuf([R, T], BF16, "obuf")

        hp = [psum([D, WW], f"hp{s}") for s in range(3)]     # 2 banks each
        panels = [psum([R, NB], f"panel{s}") for s in range(2)]

        s_w = ctx.enter_context(nc.semaphore("s_w"))
        s_wz = ctx.enter_context(nc.semaphore("s_wz"))
        s_w2p = ctx.enter_context(nc.semaphore("s_w2p"))
        s_aux = ctx.enter_context(nc.semaphore("s_aux"))
        s_x = [ctx.enter_context(nc.semaphore(f"s_x{i}")) for i in range(NXP)]
        s_mm1 = ctx.enter_context(nc.semaphore("s_mm1"))
        s_hs = ctx.enter_context(nc.semaphore("s_hs"))
        s_mm2 = ctx.enter_context(nc.semaphore("s_mm2"))
        s_pan = ctx.enter_context(nc.semaphore("s_pan"))
        s_dve = ctx.enter_context(nc.semaphore("s_dve"))
        s_sA = ctx.enter_context(nc.semaphore("s_sA"))
        s_gp = ctx.enter_context(nc.semaphore("s_gp"))
        s_eh = ctx.enter_context(nc.semaphore("s_eh"))
        s_el = ctx.enter_context(nc.semaphore("s_el"))
        s_out = ctx.enter_context(nc.semaphore("s_out"))
        block = ctx.enter_context(nc.Block())

        mAt = auxt[:, 0 * T:1 * T]
        mBrt = auxt[:, 1 * T:2 * T]
        pCHt = auxt[:, 2 * T:3 * T]
        pIVt = auxt[:, 3 * T:4 * T]

        def rev(ap, n=None, end=None):
            """Reverse view over the free dim: elements end-1, end-2, ...
            end-n (defaults: end=T, n=T)."""
            n = T if n is None else n
            end = T if end is None else end
            return bass.AP(tensor=ap.tensor, offset=ap.offset + (end - 1),
                           ap=[list(ap.ap[0]), [-1, n]])

        NGT = NBLK * NG          # total groups = 128
        NSLAB = NGT // XPG       # 32 x slabs
        SLW = XPG * GW           # slab width in xT columns

        # ---------------- SP: all DMA traffic ----------------
        @block.sync
        def _(sync):
            sync.dma_start(out=w1a[:], in_=W1[:]).then_inc(s_w, 16)
            sync.dma_start(out=w1b[:], in_=W1[:]).then_inc(s_w, 16)
            sync.dma_start(out=b1t[:], in_=b1s[:, None]).then_inc(s_w, 16)
            b2bc = bass.AP(tensor=b2.ap().tensor, offset=0, ap=[[0, D], [1, 1]])
            sync.dma_start(out=b2t[:], in_=b2bc).then_inc(s_w, 16)
            sync.dma_start(out=w2s[:], in_=W2s[:]).then_inc(s_w, 16)

            def xdma(v):
                sync.dma_start(out=xp[v % NXP][:],
                               in_=xT[:, v * SLW:(v + 1) * SLW]
                               ).then_inc(s_x[v % NXP], 16)

            for v in range(NXP):
                xdma(v)
            for v in range(NXP, NSLAB):
                # slab slot free once its previous tenant's mm1s consumed
                sync.wait_ge(s_mm1, XPG * (v - NXP) + XPG)
                xdma(v)
                if v == NXP:
                    sync.dma_start(out=auxt[:, 0:2 * T],
                                   in_=aux[:, 0:2 * T]).then_inc(s_aux, 16)
                elif v == NXP + 1:
                    sync.dma_start(out=auxt[:, 2 * T:4 * T],
                                   in_=aux[:, 2 * T:4 * T]).then_inc(s_aux, 16)
            sync.wait_ge(s_eh, 1)
            sync.dma_start(out=out[:, T2:T], in_=obuf[:, T2:T]).then_inc(s_out, 16)
            sync.wait_ge(s_el, 1)
            sync.dma_start(out=out[:, 0:T2], in_=obuf[:, 0:T2]).then_inc(s_out, 16)
            sync.wait_ge(s_out, 32)

        # ---------------- PE ----------------
        @block.tensor
        def _(tensor):
            tensor.wait_ge(s_w, 80)

            def mm2_pair(gp):
                jp, ggp = divmod(gp, NG)
                for c in range(2):
                    kp = 2 * ggp + c
                    nc.tensor.matmul(
                        out=panels[jp % 2][:],
                        lhsT=w2p[:, kp * D:(kp + 1) * D],
                        rhs=hs[gp % NHS][:, c * CW:c * CW + NB],
                        start=(ggp == 0 and c == 0),
                        stop=(ggp == NG - 1 and c == 1)).then_inc(s_mm2, 1)

            for g in range(NGT):
                v, ph = divmod(g, XPG)
                if ph == 0:
                    tensor.wait_ge(s_x[v % NXP], 16 * (v // NXP + 1))
                # WAR: hp[g%3] reused -> silu(g-3) must be done
                if g >= 3:
                    tensor.wait_ge(s_hs, g - 2)
                xbase = ph * GW
                xslot = xp[v % NXP]
                last = None
                for c in range(4):
                    po = 64 * (c & 1)          # chunks 0,2 -> rows 0:64; 1,3 -> 64:128
                    col = CW * (c >> 1)        # chunks 0,1 -> cols 0:NB; 2,3 -> CW:
                    last = nc.tensor.matmul(
                        out=hp[g % 3][po:po + 64, col:col + NB],
                        lhsT=(w1a if po == 0 else w1b)[:],
                        rhs=xslot[:, xbase + c * NB:xbase + (c + 1) * NB],
                        start=True, stop=True,
                        tile_position=(0, po))
                last.then_inc(s_mm1, 1)
                if g >= MM2_LAG:
                    gp = g - MM2_LAG
                    jp, ggp = divmod(gp, NG)
                    if g == MM2_LAG:
                        tensor.wait_ge(s_w2p, 1)
                    tensor.wait_ge(s_hs, gp + 1)
                    if ggp == 0 and jp >= 2:
                        tensor.wait_ge(s_pan, jp - 1)
                    mm2_pair(gp)
            # tail: mm2s of the last MM2_LAG groups
            for gp in range(NGT - MM2_LAG, NGT):
                tensor.wait_ge(s_hs, gp + 1)
                jp, ggp = divmod(gp, NG)
                if ggp == 0 and jp >= 2:
                    tensor.wait_ge(s_pan, jp - 1)
                mm2_pair(gp)

        # ---------------- ACT: one [128,2NB] silu per group -------------
        @block.scalar
        def _(scalar):
            func = (mybir.ActivationFunctionType.Silu if use_silu
                    else mybir.ActivationFunctionType.Sigmoid)
            scalar.wait_ge(s_w, 80)
            for g in range(NGT):
                scalar.wait_ge(s_mm1, g + 1)
                # WAR: hs[g%NHS] reused -> mm2(g-NHS) must be done
                if g >= NHS:
                    scalar.wait_ge(s_mm2, 2 * (g - NHS + 1))
                nc.scalar.activation(
                    out=hs[g % NHS][:], in_=hp[g % 3][:],
                    func=func, bias=b1t[:], scale=1.0,
                ).then_inc(s_hs, 1)

        # ---------------- GPSIMD: w2p zero-fill + hi-half epilogue ------
        @block.gpsimd
        def _(gp):
            nc.gpsimd.memset(w2p[:], 0.0).then_inc(s_wz, 1)
            # tail: finish hi half while DVE runs the second scan chunk
            gp.wait_ge(s_sA, 1)
            nc.gpsimd.tensor_mul(
                IV[:, T2:T], rev(RLr[:], n=T2, end=T2), IV[:, T2:T]
            ).then_inc(s_gp, 1)
            gp.wait_ge(s_gp, 1)
            nc.gpsimd.tensor_sub(
                obuf[:, T2:T], CH[:, T2:T], IV[:, T2:T]).then_inc(s_eh, 1)

        # ---------------- DVE: w2p fill, drains + incremental epilogue --
        @block.vector
        def _(vector):
            tick = [0]

            def step(ins):
                ins.then_inc(s_dve, 1)
                tick[0] += 1
                vector.wait_ge(s_dve, tick[0])

            # build w2p placement stationaries: block k has W2 lo at col
            # 130k (partitions 0:64) and W2 hi at col 130k+1 (64:128)
            vector.wait_ge(s_w, 80)
            vector.wait_ge(s_wz, 1)
            base_lo = w2p[0:64, 0:1]
            view_lo = bass.AP(tensor=base_lo.tensor, offset=base_lo.offset,
                              ap=[list(base_lo.ap[0]), [130, NPAIR]])
            base_hi = w2p[64:128, 1:2]
            view_hi = bass.AP(tensor=base_hi.tensor, offset=base_hi.offset,
                              ap=[list(base_hi.ap[0]), [130, NPAIR]])
            step(nc.vector.tensor_scalar_add(view_lo, view_lo, w2s[0:64, 0:1]))
            nc.vector.tensor_scalar_add(
                view_hi, view_hi, w2s[64:128, 0:1]).then_inc(s_w2p, 1)

            vector.wait_ge(s_aux, 32)
            step(nc.vector.tensor_tensor_scan(
                out=CH[:], data0=mAt, data1=pCHt,
                initial=0.0, op0=AOp.mult, op1=AOp.add))
            step(nc.vector.tensor_tensor_scan(
                out=IV[:], data0=mAt, data1=pIVt,
                initial=0.0, op0=AOp.mult, op1=AOp.add))
            step(nc.vector.tensor_scalar_add(IVp[:], IV[:], 1.0))

            for j in range(NBLK):
                lo, hi = j * NB, (j + 1) * NB
                # panel j complete after 2*NG*(j+1) mm2 incs
                vector.wait_ge(s_mm2, 2 * NG * (j + 1))
                nc.vector.tensor_scalar_add(
                    ao[:, lo:hi], panels[j % 2][:], b2t[:]
                ).then_inc(s_pan, 1)
                vector.wait_ge(s_pan, j + 1)
                # chained forward segmented scan for this block
                init = 0.0 if j == 0 else FL[:, lo - 1:lo]
                step(nc.vector.tensor_tensor_scan(
                    out=FL[:, lo:hi], data0=mAt[:, lo:hi], data1=ao[:, lo:hi],
                    initial=init, op0=AOp.mult, op1=AOp.add))
                # Q_j = (CH - FL)*IV + ao*(1+IV), accumulated into CH
                step(nc.vector.tensor_sub(CH[:, lo:hi], CH[:, lo:hi], FL[:, lo:hi]))
                step(nc.vector.tensor_mul(CH[:, lo:hi], CH[:, lo:hi], IV[:, lo:hi]))
                step(nc.vector.tensor_mul(RLr[:, lo:hi], ao[:, lo:hi], IVp[:, lo:hi]))
                step(nc.vector.tensor_add(CH[:, lo:hi], CH[:, lo:hi], RLr[:, lo:hi]))
            # tail: 2-chunk reverse scan; hi half finishes on gpsimd
            nc.vector.tensor_tensor_scan(
                out=RLr[:, 0:T2], data0=mBrt[:, 0:T2], data1=rev(ao[:], n=T2),
                initial=0.0, op0=AOp.mult, op1=AOp.add).then_inc(s_sA, 1)
            vector.wait_ge(s_sA, 1)
            step(nc.vector.tensor_tensor_scan(
                out=RLr[:, T2:T], data0=mBrt[:, T2:T],
                data1=rev(ao[:], n=T2, end=T2),
                initial=RLr[:, T2 - 1:T2], op0=AOp.mult, op1=AOp.add))
            step(nc.vector.tensor_mul(IV[:, 0:T2], rev(RLr[:], n=T2), IV[:, 0:T2]))
            nc.vector.tensor_sub(
                obuf[:, 0:T2], CH[:, 0:T2], IV[:, 0:T2]).then_inc(s_el, 1)

    return nc


def build_nc(T, use_silu=True):
    key = (T, use_silu)
    if key in _NC_CACHE:
        return _NC_CACHE[key]
    nc = build_raw(T, use_silu=use_silu)
    _NC_CACHE[key] = nc
    return nc


def _pack(batch, charge, T):
    """Pack molecules into 1024 rows of capacity T. Returns per-atom slot
    positions and the host-side mask/value grids, or None if the
    molecules don't fit."""
    n = batch.shape[0]
    sizes = np.bincount(batch, minlength=N_MOL).astype(np.int64)
    nz = np.flatnonzero(sizes)           # non-empty molecules, in order
    szs = sizes[nz]
    nrows = NCORES * R

    # greedy sequential packing of molecules into rows
    row_of = np.empty(len(nz), np.int64)
    fstart = np.empty(len(nz), np.int64)
    r, f = 0, 0
    for i, sz in enumerate(szs):
        if f + sz > T:
            r += 1
            f = 0
        row_of[i] = r
        fstart[i] = f
        f += sz
    if r >= nrows:
        return None                      # doesn't fit at this T

    slot_start = row_of * T + fstart     # global slot of each molecule start
    # per-atom global slot: atoms of molecule i occupy slot_start[i] + 0..sz
    mol_atom_start = np.concatenate([[0], np.cumsum(szs)])[:-1]
    # batch is sorted, so atom a belongs to the idx-th non-empty molecule
    idx_of_atom = np.repeat(np.arange(len(nz)), szs)
    pos_of_atom = slot_start[idx_of_atom] + (np.arange(n) - mol_atom_start[idx_of_atom])

    # masks / placed values over all rows
    fill = np.zeros(nrows, np.int64)
    np.add.at(fill, row_of, szs)
    col = np.arange(T)
    mA = np.ones((nrows, T), np.float32)
    mA.reshape(-1)[slot_start] = 0.0
    mA[col[None, :] >= fill[:, None]] = 0.0
    slot_end = slot_start + szs - 1
    mBr = np.ones((nrows, T), np.float32)
    # reversed coords: slot (r, f) -> (r, T-1-f)
    mBr.reshape(-1)[(slot_end // T) * T + (T - 1 - (slot_end % T))] = 0.0
    # pad slots in reversed coords are cols < T - fill
    mBr[col[None, :] < (T - fill[:, None])] = 0.0

    pCH = np.zeros((nrows, T), np.float32)
    pCH.reshape(-1)[slot_start] = charge[nz]
    pIV = np.zeros((nrows, T), np.float32)
    pIV.reshape(-1)[slot_start] = (1.0 / szs).astype(np.float32)

    return pos_of_atom, mA, mBr, pCH, pIV


def kernel(x_scalar, batch, charge, W1, b1, W2, b2):
    x_scalar = np.asarray(x_scalar, dtype=np.float32)
    batch = np.asarray(batch, dtype=np.int32)
    charge = np.asarray(charge, dtype=np.float32)
    W1 = np.asarray(W1, dtype=np.float32)
    b1 = np.asarray(b1, dtype=np.float32)
    W2 = np.asarray(W2, dtype=np.float32)
    b2 = np.asarray(b2, dtype=np.float32)
    n = x_scalar.shape[0]

    # tolerate unsorted batch (reference data is sorted; this is insurance)
    order = None
    if np.any(np.diff(batch) < 0):
        order = np.argsort(batch, kind="stable")
        x_scalar = x_scalar[order]
        batch = batch[order]

    T = T_TIGHT
    packed = _pack(batch, charge, T)
    if packed is None:
        T = T_SAFE
        packed = _pack(batch, charge, T)
        assert packed is not None, "molecules do not fit even at T=2048"
    pos_of_atom, mA, mBr, pCH, pIV = packed
    NB = T // 4
    NPAIR = R // 2
    NBLK = 4
    S = R * T

    # padded, packed, transposed x per core (bf16)
    xpad = np.zeros((NCORES * S, D), NP_BF16)
    xpad[pos_of_atom] = x_scalar.astype(NP_BF16)
    xT_cores = []
    for c in range(NCORES):
        a = xpad[c * S:(c + 1) * S].reshape(NPAIR, 2, NBLK, NB, D)
        a = a.transpose(2, 0, 1, 3, 4).reshape(S, D)   # j-major stream order
        xT_cores.append(np.ascontiguousarray(a.T))
    del xpad

    W2s = np.concatenate([W2[:, 0], W2[:, 0]]).reshape(D, 1).astype(np.float32)
    W1 = W1.astype(NP_BF16)
    b1s = np.concatenate([b1, b1]).astype(np.float32)

    nc = build_nc(T, use_silu=True)
    in_maps = []
    for c in range(NCORES):
        sl = slice(c * R, (c + 1) * R)
        auxc = np.concatenate([mA[sl], mBr[sl], pCH[sl], pIV[sl]],
                              axis=1).astype(NP_BF16)
        in_maps.append({
            "xT": xT_cores[c], "W1": W1, "b1s": b1s, "b2": b2, "W2s": W2s,
            "aux": np.ascontiguousarray(auxc),
        })

    import os
    trace = bool(int(os.environ.get("ATOMIC_TRACE", "0")))
    res = run_bass_kernel_spmd(nc, in_maps, list(range(NCORES)), trace=trace)
    LAST_RUN_INFO["exec_time_ns"] = getattr(res, "exec_time_ns", None)
    LAST_RUN_INFO["profile_json"] = getattr(res, "profile_json", None)

    big = np.concatenate([res.results[c]["out"].reshape(-1).astype(np.float32)
                          for c in range(NCORES)])
    at = big[pos_of_atom]
    if order is not None:
        inv = np.empty_like(order)
        inv[order] = np.arange(n)
        at = at[inv]
    return at


# revision 7
# speedup vs baseline: 1.1865x; 1.1865x over previous
"""AtomicCharge Trainium2 kernel (nn_AtomicCharge_77781857730661).

Strategy
--------
Data-parallel over atoms across 8 NeuronCores. The host packs molecules
(contiguous runs of the sorted `batch` tensor) into 1024 partition-rows
(8 cores x 128 partitions) of capacity T slots (T=1984 when the data
packs that tight, else 2048), so every molecule lives contiguously
along the free dim of one partition. x is uploaded pre-transposed in
bf16 (halves HBM traffic vs fp32), in the j-major order the device
pipeline streams it. aux masks travel bf16; output returns bf16.

Per core (raw bass, explicit semaphores; waits are standalone):
  PE:  per group (4 chunks x NB atoms): 4x mm1 (W1^T x, bf16) col-tiled
       2x on the PE array -- chunks 0/2 via tile (0,0) into hp[0:64],
       chunks 1/3 via tile (0,64) into hp[64:128]; consecutive
       tile-0/tile-64 matmuls stream concurrently into one [128,2NB]
       PSUM tile per group. Then 2x K=128 mm2 with per-pair placement
       stationaries accumulate atom_out into the packed [128,NB] panel
       PSUM for block j (panels double-buffered; mm2 lagged 6 groups).
       The mm2 placement stationary w2p is BUILT ON DEVICE (gpsimd
       memset + 2 strided DVE broadcast-adds) instead of DMAing 2.1MB.
  ACT: ONE [128,2NB] Silu per group (bias b1 fused, bf16 out).
  DVE: per-j panel drains (+b2), chained per-block forward segmented
       scans and Q = (CH-FL)*IV + ao*(1+IV) precompute; the tail is a
       2-chunk reverse scan with the hi-half elementwise finish on
       GPSIMD in parallel, and the output DMAs per half:
       out = Q - rev(RL)*IV.

Pipelining: x slabs 4 groups each (~16KB/partition DMA lines), 4-deep;
hp PSUM triple-buffered; hs 8-deep.
The compile enables walrus redundant-LDWEIGHTS elimination.
HW: ~200 us/core on trn2 (8 cores); rel err ~4e-3 vs fp32 ref (bf16).
"""
import sys

sys.path.insert(0, "/opt/trn_rl_repo")

import numpy as np
import ml_dtypes

import concourse.bass as bass
from concourse import mybir
from concourse.bass_utils import run_bass_kernel_spmd
import concourse.bass_utils as _bu

# Enable walrus's redundant-LDWEIGHTS elimination (off by default in this
# stack); our mm1s reuse the same stationaries within a group.
if not getattr(_bu, "_ldwopt_patched", False):
    _orig_run_command = _bu.run_command

    def _run_command_ldwopt(argv, **kw):
        argv = [a.replace("--enable-ldw-opt=false", "--enable-ldw-opt=true")
                for a in argv]
        return _orig_run_command(argv, **kw)

    _bu.run_command = _run_command_ldwopt
    _bu._ldwopt_patched = True

F32 = mybir.dt.float32
BF16 = mybir.dt.bfloat16
NP_BF16 = ml_dtypes.bfloat16

# problem constants (hardcoded per spec)
N_ATOMS = 2_000_000
N_MOL = 50_000
D = 128      # node feature dim = SBUF partitions
H = 64       # hidden dim
NCORES = 8
R = 128      # atom-layout rows per core (partitions)
T_TIGHT = 1984
T_SAFE = 2048

_NC_CACHE = {}
LAST_RUN_INFO = {}


def build_raw(T, use_silu=True):
    """j-outer pipeline: groups of 2 pairs; col-tiled mm1 into one
    [128,2NB] PSUM tile; one [128,2NB] silu; K=128 mm2; panels
    double-buffered across j; incremental DVE epilogue."""
    NB = T // 4
    NPAIR = R // 2
    NBLK = 4
    NG = NPAIR // 2             # groups per block; group = 2 pairs = 4*NB atoms
    S = R * T
    CW = 512                    # chunk stride inside hp/hs (bank-aligned)
    WW = 2 * CW                 # hs width per group (pad cols beyond NB unused)
    GW = 4 * NB                 # xT columns per group (tight, no padding)
    XPG = 4                     # groups per x slab
    NXP = 5                     # x slab buffers
    NHS = 10                    # hs buffers
    MM2_LAG = 6                 # mm2 trails mm1 by this many groups
    T2 = T // 2
    AOp = mybir.AluOpType

    nc = bass.Bass()
    # xT is laid out j-major on the host: block j, then pair k, then
    # (row 2k | row 2k+1) x NB columns
    xT = nc.declare_dram_parameter("xT", [D, S], BF16, isOutput=False)
    W1 = nc.declare_dram_parameter("W1", [D, H], BF16, isOutput=False)
    b1s = nc.declare_dram_parameter("b1s", [D], F32, isOutput=False)
    b2 = nc.declare_dram_parameter("b2", [1], F32, isOutput=False)
    W2s = nc.declare_dram_parameter("W2s", [D, 1], F32, isOutput=False)
    aux = nc.declare_dram_parameter("aux", [R, 4 * T], BF16, isOutput=False)
    out = nc.declare_dram_parameter("out", [R, T], BF16, isOutput=True)

    from contextlib import ExitStack
    with ExitStack() as ctx:
        def sbuf(shape, dtype, name):
            return ctx.enter_context(nc.sbuf_tensor(name, shape, dtype))

        def psum(shape, name):
            return ctx.enter_context(nc.psum_tensor(name, shape, F32))

        w1a = sbuf([D, H], BF16, "w1a")
        w1b = sbuf([D, H], BF16, "w1b")
        b1t = sbuf([D, 1], F32, "b1t")
        b2t = sbuf([D, 1], F32, "b2t")
        w2s = sbuf([D, 1], F32, "w2s")
        w2p = sbuf([D, NPAIR * D], BF16, "w2p")
        auxt = sbuf([R, 4 * T], BF16, "auxt")
        xp = [sbuf([D, XPG * GW], BF16, f"xp{s}") for s in range(NXP)]
        hs = [sbuf([D, WW], BF16, f"hs{s}") for s in range(NHS)]
        ao = sbuf([R, T], F32, "ao")
        FL = sbuf([R, T], F32, "FL")
        RLr = sbuf([R, T], F32, "RLr")
        CH = sbuf([R, T], F32, "CH")
        IV = sbuf([R, T], F32, "IV")
        IVp = sbuf([R, T], F32, "IVp")
        obuf = sbuf([R, T], BF16, "obuf")

        hp = [psum([D, WW], f"hp{s}") for s in range(3)]     # 2 banks each
        panels = [psum([R, NB], f"panel{s}") for s in range(2)]

        s_w = ctx.enter_context(nc.semaphore("s_w"))
        s_wz = ctx.enter_context(nc.semaphore("s_wz"))
        s_w2p = ctx.enter_context(nc.semaphore("s_w2p"))
        s_aux = ctx.enter_context(nc.semaphore("s_aux"))
        s_x = [ctx.enter_context(nc.semaphore(f"s_x{i}")) for i in range(NXP)]
        s_mm1 = ctx.enter_context(nc.semaphore("s_mm1"))
        s_hs = ctx.enter_context(nc.semaphore("s_hs"))
        s_mm2 = ctx.enter_context(nc.semaphore("s_mm2"))
        s_pan = ctx.enter_context(nc.semaphore("s_pan"))
        s_dve = ctx.enter_context(nc.semaphore("s_dve"))
        s_sA = ctx.enter_context(nc.semaphore("s_sA"))
        s_gp = ctx.enter_context(nc.semaphore("s_gp"))
        s_eh = ctx.enter_context(nc.semaphore("s_eh"))
        s_el = ctx.enter_context(nc.semaphore("s_el"))
        s_out = ctx.enter_context(nc.semaphore("s_out"))
        block = ctx.enter_context(nc.Block())

        mAt = auxt[:, 0 * T:1 * T]
        mBrt = auxt[:, 1 * T:2 * T]
        pCHt = auxt[:, 2 * T:3 * T]
        pIVt = auxt[:, 3 * T:4 * T]

        def rev(ap, n=None, end=None):
            """Reverse view over the free dim: elements end-1, end-2, ...
            end-n (defaults: end=T, n=T)."""
            n = T if n is None else n
            end = T if end is None else end
            return bass.AP(tensor=ap.tensor, offset=ap.offset + (end - 1),
                           ap=[list(ap.ap[0]), [-1, n]])

        NGT = NBLK * NG          # total groups = 128
        NSLAB = NGT // XPG       # 32 x slabs
        SLW = XPG * GW           # slab width in xT columns

        # ---------------- SP: all DMA traffic ----------------
        @block.sync
        def _(sync):
            sync.dma_start(out=w1a[:], in_=W1[:]).then_inc(s_w, 16)
            sync.dma_start(out=w1b[:], in_=W1[:]).then_inc(s_w, 16)
            sync.dma_start(out=b1t[:], in_=b1s[:, None]).then_inc(s_w, 16)
            b2bc = bass.AP(tensor=b2.ap().tensor, offset=0, ap=[[0, D], [1, 1]])
            sync.dma_start(out=b2t[:], in_=b2bc).then_inc(s_w, 16)
            sync.dma_start(out=w2s[:], in_=W2s[:]).then_inc(s_w, 16)

            def xdma(v):
                sync.dma_start(out=xp[v % NXP][:],
                               in_=xT[:, v * SLW:(v + 1) * SLW]
                               ).then_inc(s_x[v % NXP], 16)

            for v in range(NXP):
                xdma(v)
            for v in range(NXP, NSLAB):
                # slab slot free once its previous tenant's mm1s consumed
                sync.wait_ge(s_mm1, XPG * (v - NXP) + XPG)
                xdma(v)
                if v == NXP:
                    sync.dma_start(out=auxt[:, 0:2 * T],
                                   in_=aux[:, 0:2 * T]).then_inc(s_aux, 16)
                elif v == NXP + 1:
                    sync.dma_start(out=auxt[:, 2 * T:4 * T],
                                   in_=aux[:, 2 * T:4 * T]).then_inc(s_aux, 16)
            sync.wait_ge(s_eh, 1)
            sync.dma_start(out=out[:, T2:T], in_=obuf[:, T2:T]).then_inc(s_out, 16)
            sync.wait_ge(s_el, 1)
            sync.dma_start(out=out[:, 0:T2], in_=obuf[:, 0:T2]).then_inc(s_out, 16)
            sync.wait_ge(s_out, 32)

        # ---------------- PE ----------------
        @block.tensor
        def _(tensor):
            tensor.wait_ge(s_w, 80)

            def mm2_pair(gp):
                jp, ggp = divmod(gp, NG)
                for c in range(2):
                    kp = 2 * ggp + c
                    nc.tensor.matmul(
                        out=panels[jp % 2][:],
                        lhsT=w2p[:, kp * D:(kp + 1) * D],
                        rhs=hs[gp % NHS][:, c * CW:c * CW + NB],
                        start=(ggp == 0 and c == 0),
                        stop=(ggp == NG - 1 and c == 1)).then_inc(s_mm2, 1)

            for g in range(NGT):
                v, ph = divmod(g, XPG)
                if ph == 0:
                    tensor.wait_ge(s_x[v % NXP], 16 * (v // NXP + 1))
                # WAR: hp[g%3] reused -> silu(g-3) must be done
                if g >= 3:
                    tensor.wait_ge(s_hs, g - 2)
                xbase = ph * GW
                xslot = xp[v % NXP]
                last = None
                for c in range(4):
                    po = 64 * (c & 1)          # chunks 0,2 -> rows 0:64; 1,3 -> 64:128
                    col = CW * (c >> 1)        # chunks 0,1 -> cols 0:NB; 2,3 -> CW:
                    last = nc.tensor.matmul(
                        out=hp[g % 3][po:po + 64, col:col + NB],
                        lhsT=(w1a if po == 0 else w1b)[:],
                        rhs=xslot[:, xbase + c * NB:xbase + (c + 1) * NB],
                        start=True, stop=True,
                        tile_position=(0, po))
                last.then_inc(s_mm1, 1)
                if g >= MM2_LAG:
                    gp = g - MM2_LAG
                    jp, ggp = divmod(gp, NG)
                    if g == MM2_LAG:
                        tensor.wait_ge(s_w2p, 1)
                    tensor.wait_ge(s_hs, gp + 1)
                    if ggp == 0 and jp >= 2:
                        tensor.wait_ge(s_pan, jp - 1)
                    mm2_pair(gp)
            # tail: mm2s of the last MM2_LAG groups
            for gp in range(NGT - MM2_LAG, NGT):
                tensor.wait_ge(s_hs, gp + 1)
                jp, ggp = divmod(gp, NG)
                if ggp == 0 and jp >= 2:
                    tensor.wait_ge(s_pan, jp - 1)
                mm2_pair(gp)

        # ---------------- ACT: one [128,2NB] silu per group -------------
        @block.scalar
        def _(scalar):
            func = (mybir.ActivationFunctionType.Silu if use_silu
                    else mybir.ActivationFunctionType.Sigmoid)
            scalar.wait_ge(s_w, 80)
            for g in range(NGT):
                scalar.wait_ge(s_mm1, g + 1)
                # WAR: hs[g%NHS] reused -> mm2(g-NHS) must be done
                if g >= NHS:
                    scalar.wait_ge(s_mm2, 2 * (g - NHS + 1))
                nc.scalar.activation(
                    out=hs[g % NHS][:], in_=hp[g % 3][:],
                    func=func, bias=b1t[:], scale=1.0,
                ).then_inc(s_hs, 1)

        # ---------------- GPSIMD: w2p zero-fill ------------------------
        @block.gpsimd
        def _(gp):
            nc.gpsimd.memset(w2p[:], 0.0).then_inc(s_wz, 1)

        # ---------------- DVE: w2p fill, drains + incremental epilogue --
        @block.vector
        def _(vector):
            tick = [0]

            def step(ins):
                ins.then_inc(s_dve, 1)
                tick[0] += 1
                vector.wait_ge(s_dve, tick[0])

            # build w2p placement stationaries: block k has W2 lo at col
            # 130k (partitions 0:64) and W2 hi at col 130k+1 (64:128)
            vector.wait_ge(s_w, 80)
            vector.wait_ge(s_wz, 1)
            base_lo = w2p[0:64, 0:1]
            view_lo = bass.AP(tensor=base_lo.tensor, offset=base_lo.offset,
                              ap=[list(base_lo.ap[0]), [130, NPAIR]])
            base_hi = w2p[64:128, 1:2]
            view_hi = bass.AP(tensor=base_hi.tensor, offset=base_hi.offset,
                              ap=[list(base_hi.ap[0]), [130, NPAIR]])
            step(nc.vector.tensor_scalar_add(view_lo, view_lo, w2s[0:64, 0:1]))
            nc.vector.tensor_scalar_add(
                view_hi, view_hi, w2s[64:128, 0:1]).then_inc(s_w2p, 1)

            vector.wait_ge(s_aux, 32)
            step(nc.vector.tensor_tensor_scan(
                out=CH[:], data0=mAt, data1=pCHt,
                initial=0.0, op0=AOp.mult, op1=AOp.add))
            step(nc.vector.tensor_tensor_scan(
                out=IV[:], data0=mAt, data1=pIVt,
                initial=0.0, op0=AOp.mult, op1=AOp.add))
            step(nc.vector.tensor_scalar_add(IVp[:], IV[:], 1.0))

            for j in range(NBLK):
                lo, hi = j * NB, (j + 1) * NB
                # panel j complete after 2*NG*(j+1) mm2 incs
                vector.wait_ge(s_mm2, 2 * NG * (j + 1))
                nc.vector.tensor_scalar_add(
                    ao[:, lo:hi], panels[j % 2][:], b2t[:]
                ).then_inc(s_pan, 1)
                vector.wait_ge(s_pan, j + 1)
                # chained forward segmented scan for this block
                init = 0.0 if j == 0 else FL[:, lo - 1:lo]
                step(nc.vector.tensor_tensor_scan(
                    out=FL[:, lo:hi], data0=mAt[:, lo:hi], data1=ao[:, lo:hi],
                    initial=init, op0=AOp.mult, op1=AOp.add))
                # Q_j = (CH - FL)*IV + ao*(1+IV), accumulated into CH
                step(nc.vector.tensor_sub(CH[:, lo:hi], CH[:, lo:hi], FL[:, lo:hi]))
                step(nc.vector.tensor_mul(CH[:, lo:hi], CH[:, lo:hi], IV[:, lo:hi]))
                step(nc.vector.tensor_mul(RLr[:, lo:hi], ao[:, lo:hi], IVp[:, lo:hi]))
                step(nc.vector.tensor_add(CH[:, lo:hi], CH[:, lo:hi], RLr[:, lo:hi]))
            # tail: 2-chunk reverse scan; hi-half output DMAs while the
            # lo half is still being computed
            step(nc.vector.tensor_tensor_scan(
                out=RLr[:, 0:T2], data0=mBrt[:, 0:T2], data1=rev(ao[:], n=T2),
                initial=0.0, op0=AOp.mult, op1=AOp.add))
            step(nc.vector.tensor_mul(
                IV[:, T2:T], rev(RLr[:], n=T2, end=T2), IV[:, T2:T]))
            nc.vector.tensor_sub(
                obuf[:, T2:T], CH[:, T2:T], IV[:, T2:T]).then_inc(s_eh, 1)
            step(nc.vector.tensor_tensor_scan(
                out=RLr[:, T2:T], data0=mBrt[:, T2:T],
                data1=rev(ao[:], n=T2, end=T2),
                initial=RLr[:, T2 - 1:T2], op0=AOp.mult, op1=AOp.add))
            step(nc.vector.tensor_mul(IV[:, 0:T2], rev(RLr[:], n=T2), IV[:, 0:T2]))
            nc.vector.tensor_sub(
                obuf[:, 0:T2], CH[:, 0:T2], IV[:, 0:T2]).then_inc(s_el, 1)

    return nc


def build_nc(T, use_silu=True):
    key = (T, use_silu)
    if key in _NC_CACHE:
        return _NC_CACHE[key]
    nc = build_raw(T, use_silu=use_silu)
    _NC_CACHE[key] = nc
    return nc


def _pack(batch, charge, T):
    """Pack molecules into 1024 rows of capacity T. Returns per-atom slot
    positions and the host-side mask/value grids, or None if the
    molecules don't fit."""
    n = batch.shape[0]
    sizes = np.bincount(batch, minlength=N_MOL).astype(np.int64)
    nz = np.flatnonzero(sizes)           # non-empty molecules, in order
    szs = sizes[nz]
    nrows = NCORES * R

    # greedy sequential packing of molecules into rows
    row_of = np.empty(len(nz), np.int64)
    fstart = np.empty(len(nz), np.int64)
    r, f = 0, 0
    for i, sz in enumerate(szs):
        if f + sz > T:
            r += 1
            f = 0
        row_of[i] = r
        fstart[i] = f
        f += sz
    if r >= nrows:
        return None                      # doesn't fit at this T

    slot_start = row_of * T + fstart     # global slot of each molecule start
    # per-atom global slot: atoms of molecule i occupy slot_start[i] + 0..sz
    mol_atom_start = np.concatenate([[0], np.cumsum(szs)])[:-1]
    # batch is sorted, so atom a belongs to the idx-th non-empty molecule
    idx_of_atom = np.repeat(np.arange(len(nz)), szs)
    pos_of_atom = slot_start[idx_of_atom] + (np.arange(n) - mol_atom_start[idx_of_atom])

    # masks / placed values over all rows
    fill = np.zeros(nrows, np.int64)
    np.add.at(fill, row_of, szs)
    col = np.arange(T)
    mA = np.ones((nrows, T), np.float32)
    mA.reshape(-1)[slot_start] = 0.0
    mA[col[None, :] >= fill[:, None]] = 0.0
    slot_end = slot_start + szs - 1
    mBr = np.ones((nrows, T), np.float32)
    # reversed coords: slot (r, f) -> (r, T-1-f)
    mBr.reshape(-1)[(slot_end // T) * T + (T - 1 - (slot_end % T))] = 0.0
    # pad slots in reversed coords are cols < T - fill
    mBr[col[None, :] < (T - fill[:, None])] = 0.0

    pCH = np.zeros((nrows, T), np.float32)
    pCH.reshape(-1)[slot_start] = charge[nz]
    pIV = np.zeros((nrows, T), np.float32)
    pIV.reshape(-1)[slot_start] = (1.0 / szs).astype(np.float32)

    return pos_of_atom, mA, mBr, pCH, pIV


def kernel(x_scalar, batch, charge, W1, b1, W2, b2):
    x_scalar = np.asarray(x_scalar, dtype=np.float32)
    batch = np.asarray(batch, dtype=np.int32)
    charge = np.asarray(charge, dtype=np.float32)
    W1 = np.asarray(W1, dtype=np.float32)
    b1 = np.asarray(b1, dtype=np.float32)
    W2 = np.asarray(W2, dtype=np.float32)
    b2 = np.asarray(b2, dtype=np.float32)
    n = x_scalar.shape[0]

    # tolerate unsorted batch (reference data is sorted; this is insurance)
    order = None
    if np.any(np.diff(batch) < 0):
        order = np.argsort(batch, kind="stable")
        x_scalar = x_scalar[order]
        batch = batch[order]

    T = T_TIGHT
    packed = _pack(batch, charge, T)
    if packed is None:
        T = T_SAFE
        packed = _pack(batch, charge, T)
        assert packed is not None, "molecules do not fit even at T=2048"
    pos_of_atom, mA, mBr, pCH, pIV = packed
    NB = T // 4
    NPAIR = R // 2
    NBLK = 4
    S = R * T

    # padded, packed, transposed x per core (bf16)
    xpad = np.zeros((NCORES * S, D), NP_BF16)
    xpad[pos_of_atom] = x_scalar.astype(NP_BF16)
    xT_cores = []
    for c in range(NCORES):
        a = xpad[c * S:(c + 1) * S].reshape(NPAIR, 2, NBLK, NB, D)
        a = a.transpose(2, 0, 1, 3, 4).reshape(S, D)   # j-major stream order
        xT_cores.append(np.ascontiguousarray(a.T))
    del xpad

    W2s = np.concatenate([W2[:, 0], W2[:, 0]]).reshape(D, 1).astype(np.float32)
    W1 = W1.astype(NP_BF16)
    b1s = np.concatenate([b1, b1]).astype(np.float32)

    nc = build_nc(T, use_silu=True)
    in_maps = []
    for c in range(NCORES):
        sl = slice(c * R, (c + 1) * R)
        auxc = np.concatenate([mA[sl], mBr[sl], pCH[sl], pIV[sl]],
                              axis=1).astype(NP_BF16)
        in_maps.append({
            "xT": xT_cores[c], "W1": W1, "b1s": b1s, "b2": b2, "W2s": W2s,
            "aux": np.ascontiguousarray(auxc),
        })

    import os
    trace = bool(int(os.environ.get("ATOMIC_TRACE", "0")))
    res = run_bass_kernel_spmd(nc, in_maps, list(range(NCORES)), trace=trace)
    LAST_RUN_INFO["exec_time_ns"] = getattr(res, "exec_time_ns", None)
    LAST_RUN_INFO["profile_json"] = getattr(res, "profile_json", None)

    big = np.concatenate([res.results[c]["out"].reshape(-1).astype(np.float32)
                          for c in range(NCORES)])
    at = big[pos_of_atom]
    if order is not None:
        inv = np.empty_like(order)
        inv[order] = np.arange(n)
        at = at[inv]
    return at


# revision 8
# speedup vs baseline: 1.1953x; 1.0074x over previous
"""AtomicCharge Trainium2 kernel (nn_AtomicCharge_77781857730661).

Strategy
--------
Data-parallel over atoms across 8 NeuronCores. The host packs molecules
(contiguous runs of the sorted `batch` tensor) into 1024 partition-rows
(8 cores x 128 partitions) of capacity T slots (T=1984 when the data
packs that tight, else 2048), so every molecule lives contiguously
along the free dim of one partition. x is uploaded pre-transposed in
bf16 (halves HBM traffic vs fp32), in the j-major order the device
pipeline streams it. aux masks travel bf16; output returns bf16.

Per core (raw bass, explicit semaphores; waits are standalone):
  PE:  per group (4 chunks x NB atoms): 4x mm1 (W1^T x, bf16) col-tiled
       2x on the PE array -- chunks 0/2 via tile (0,0) into hp[0:64],
       chunks 1/3 via tile (0,64) into hp[64:128]; consecutive
       tile-0/tile-64 matmuls stream concurrently into one [128,2NB]
       PSUM tile per group. Then 2x K=128 mm2 with per-pair placement
       stationaries accumulate atom_out into the packed [128,NB] panel
       PSUM for block j (panels double-buffered; mm2 lagged 6 groups).
       The mm2 placement stationary w2p is BUILT ON DEVICE (gpsimd
       memset + 2 strided DVE broadcast-adds) instead of DMAing 2.1MB.
  ACT: ONE [128,2NB] Silu per group (bias b1 fused, bf16 out).
  DVE: per-j panel drains (+b2), chained per-block forward segmented
       scans and Q = (CH-FL)*IV + ao*(1+IV) precompute; the tail is a
       2-chunk reverse scan with the hi-half elementwise finish on
       GPSIMD in parallel, and the output DMAs per half:
       out = Q - rev(RL)*IV.

Pipelining: x slabs 4 groups each (~16KB/partition DMA lines), 4-deep;
hp PSUM triple-buffered; hs 8-deep.
The compile enables walrus redundant-LDWEIGHTS elimination.
HW: ~200 us/core on trn2 (8 cores); rel err ~4e-3 vs fp32 ref (bf16).
"""
import sys

sys.path.insert(0, "/opt/trn_rl_repo")

import numpy as np
import ml_dtypes

import concourse.bass as bass
from concourse import mybir
from concourse.bass_utils import run_bass_kernel_spmd
import concourse.bass_utils as _bu

# Enable walrus's redundant-LDWEIGHTS elimination (off by default in this
# stack); our mm1s reuse the same stationaries within a group.
if not getattr(_bu, "_ldwopt_patched", False):
    _orig_run_command = _bu.run_command

    def _run_command_ldwopt(argv, **kw):
        argv = [a.replace("--enable-ldw-opt=false", "--enable-ldw-opt=true")
                for a in argv]
        return _orig_run_command(argv, **kw)

    _bu.run_command = _run_command_ldwopt
    _bu._ldwopt_patched = True

F32 = mybir.dt.float32
BF16 = mybir.dt.bfloat16
NP_BF16 = ml_dtypes.bfloat16

# problem constants (hardcoded per spec)
N_ATOMS = 2_000_000
N_MOL = 50_000
D = 128      # node feature dim = SBUF partitions
H = 64       # hidden dim
NCORES = 8
R = 128      # atom-layout rows per core (partitions)
T_TIGHT = 1984
T_SAFE = 2048

_NC_CACHE = {}
LAST_RUN_INFO = {}


def build_raw(T, use_silu=True):
    """j-outer pipeline: groups of 2 pairs; col-tiled mm1 into one
    [128,2NB] PSUM tile; one [128,2NB] silu; K=128 mm2; panels
    double-buffered across j; incremental DVE epilogue."""
    NB = T // 4
    NPAIR = R // 2
    NBLK = 4
    NG = NPAIR // 2             # groups per block; group = 2 pairs = 4*NB atoms
    S = R * T
    CW = 512                    # chunk stride inside hp/hs (bank-aligned)
    WW = 2 * CW                 # hs width per group (pad cols beyond NB unused)
    GW = 4 * NB                 # xT columns per group (tight, no padding)
    XPG = 4                     # groups per x slab
    NXP = 5                     # x slab buffers
    NHS = 10                    # hs buffers
    MM2_LAG = 6                 # mm2 trails mm1 by this many groups
    T2 = T // 2
    AOp = mybir.AluOpType

    nc = bass.Bass()
    # xT is laid out j-major on the host: block j, then pair k, then
    # (row 2k | row 2k+1) x NB columns
    xT = nc.declare_dram_parameter("xT", [D, S], BF16, isOutput=False)
    W1 = nc.declare_dram_parameter("W1", [D, H], BF16, isOutput=False)
    b1s = nc.declare_dram_parameter("b1s", [D], F32, isOutput=False)
    b2 = nc.declare_dram_parameter("b2", [1], F32, isOutput=False)
    W2s = nc.declare_dram_parameter("W2s", [D, 1], F32, isOutput=False)
    aux = nc.declare_dram_parameter("aux", [R, 4 * T], BF16, isOutput=False)
    out = nc.declare_dram_parameter("out", [R, T], BF16, isOutput=True)

    from contextlib import ExitStack
    with ExitStack() as ctx:
        def sbuf(shape, dtype, name):
            return ctx.enter_context(nc.sbuf_tensor(name, shape, dtype))

        def psum(shape, name):
            return ctx.enter_context(nc.psum_tensor(name, shape, F32))

        w1a = sbuf([D, H], BF16, "w1a")
        w1b = sbuf([D, H], BF16, "w1b")
        b1t = sbuf([D, 1], F32, "b1t")
        b2t = sbuf([D, 1], F32, "b2t")
        w2s = sbuf([D, 1], F32, "w2s")
        w2p = sbuf([D, NPAIR * D], BF16, "w2p")
        auxt = sbuf([R, 4 * T], BF16, "auxt")
        xp = [sbuf([D, XPG * GW], BF16, f"xp{s}") for s in range(NXP)]
        hs = [sbuf([D, WW], BF16, f"hs{s}") for s in range(NHS)]
        ao = sbuf([R, T], F32, "ao")
        FL = sbuf([R, T], F32, "FL")
        RLr = sbuf([R, T], F32, "RLr")
        CH = sbuf([R, T], F32, "CH")
        IV = sbuf([R, T], F32, "IV")
        IVp = sbuf([R, T], F32, "IVp")
        obuf = sbuf([R, T], BF16, "obuf")

        hp = [psum([D, WW], f"hp{s}") for s in range(3)]     # 2 banks each
        panels = [psum([R, NB], f"panel{s}") for s in range(2)]

        s_w = ctx.enter_context(nc.semaphore("s_w"))
        s_wz = ctx.enter_context(nc.semaphore("s_wz"))
        s_w2p = ctx.enter_context(nc.semaphore("s_w2p"))
        s_aux = ctx.enter_context(nc.semaphore("s_aux"))
        s_x = [ctx.enter_context(nc.semaphore(f"s_x{i}")) for i in range(NXP)]
        s_mm1 = ctx.enter_context(nc.semaphore("s_mm1"))
        s_hs = ctx.enter_context(nc.semaphore("s_hs"))
        s_mm2 = ctx.enter_context(nc.semaphore("s_mm2"))
        s_pan = ctx.enter_context(nc.semaphore("s_pan"))
        s_dve = ctx.enter_context(nc.semaphore("s_dve"))
        s_sA = ctx.enter_context(nc.semaphore("s_sA"))
        s_gp = ctx.enter_context(nc.semaphore("s_gp"))
        s_eh = ctx.enter_context(nc.semaphore("s_eh"))
        s_el = ctx.enter_context(nc.semaphore("s_el"))
        s_out = ctx.enter_context(nc.semaphore("s_out"))
        block = ctx.enter_context(nc.Block())

        mAt = auxt[:, 0 * T:1 * T]
        mBrt = auxt[:, 1 * T:2 * T]
        pCHt = auxt[:, 2 * T:3 * T]
        pIVt = auxt[:, 3 * T:4 * T]

        def rev(ap, n=None, end=None):
            """Reverse view over the free dim: elements end-1, end-2, ...
            end-n (defaults: end=T, n=T)."""
            n = T if n is None else n
            end = T if end is None else end
            return bass.AP(tensor=ap.tensor, offset=ap.offset + (end - 1),
                           ap=[list(ap.ap[0]), [-1, n]])

        NGT = NBLK * NG          # total groups = 128
        NSLAB = NGT // XPG       # 32 x slabs
        SLW = XPG * GW           # slab width in xT columns

        # slab plan: 4-group slabs, then single-group slabs at the end so
        # the final groups pipeline through PE/ACT instead of arriving as
        # one 4-group burst
        SLABS = [(4 * i, 4) for i in range(30)] + [(120 + i, 1) for i in range(8)]
        slab_of_group = {}
        for i, (st, n) in enumerate(SLABS):
            for k in range(n):
                slab_of_group[st + k] = i

        # ---------------- SP: all DMA traffic ----------------
        @block.sync
        def _(sync):
            def xdma(i):
                st, n = SLABS[i]
                sync.dma_start(out=xp[i % NXP][:, 0:n * GW],
                               in_=xT[:, st * GW:(st + n) * GW]
                               ).then_inc(s_x[i % NXP], 16)

            xdma(0)
            sync.dma_start(out=w1a[:], in_=W1[:]).then_inc(s_w, 16)
            sync.dma_start(out=w1b[:], in_=W1[:]).then_inc(s_w, 16)
            sync.dma_start(out=b1t[:], in_=b1s[:, None]).then_inc(s_w, 16)
            b2bc = bass.AP(tensor=b2.ap().tensor, offset=0, ap=[[0, D], [1, 1]])
            sync.dma_start(out=b2t[:], in_=b2bc).then_inc(s_w, 16)
            sync.dma_start(out=w2s[:], in_=W2s[:]).then_inc(s_w, 16)
            for i in range(1, NXP):
                xdma(i)
            for i in range(NXP, len(SLABS)):
                # slab slot free once its previous tenant's mm1s consumed
                pst, pn = SLABS[i - NXP]
                sync.wait_ge(s_mm1, pst + pn)
                xdma(i)
                if i == NXP:
                    sync.dma_start(out=auxt[:, 0:2 * T],
                                   in_=aux[:, 0:2 * T]).then_inc(s_aux, 16)
                elif i == NXP + 1:
                    sync.dma_start(out=auxt[:, 2 * T:4 * T],
                                   in_=aux[:, 2 * T:4 * T]).then_inc(s_aux, 16)
            sync.wait_ge(s_eh, 1)
            sync.dma_start(out=out[:, T2:T], in_=obuf[:, T2:T]).then_inc(s_out, 16)
            sync.wait_ge(s_el, 1)
            sync.dma_start(out=out[:, 0:T2], in_=obuf[:, 0:T2]).then_inc(s_out, 16)
            sync.wait_ge(s_out, 32)

        # ---------------- PE ----------------
        @block.tensor
        def _(tensor):
            tensor.wait_ge(s_w, 80)

            def mm2_pair(gp):
                jp, ggp = divmod(gp, NG)
                for c in range(2):
                    kp = 2 * ggp + c
                    nc.tensor.matmul(
                        out=panels[jp % 2][:],
                        lhsT=w2p[:, kp * D:(kp + 1) * D],
                        rhs=hs[gp % NHS][:, c * CW:c * CW + NB],
                        start=(ggp == 0 and c == 0),
                        stop=(ggp == NG - 1 and c == 1)).then_inc(s_mm2, 1)

            for g in range(NGT):
                i = slab_of_group[g]
                st, n = SLABS[i]
                if g == st:
                    tensor.wait_ge(s_x[i % NXP], 16 * (i // NXP + 1))
                # WAR: hp[g%3] reused -> silu(g-3) must be done
                if g >= 3:
                    tensor.wait_ge(s_hs, g - 2)
                xbase = (g - st) * GW
                xslot = xp[i % NXP]
                last = None
                for c in range(4):
                    po = 64 * (c & 1)          # chunks 0,2 -> rows 0:64; 1,3 -> 64:128
                    col = CW * (c >> 1)        # chunks 0,1 -> cols 0:NB; 2,3 -> CW:
                    last = nc.tensor.matmul(
                        out=hp[g % 3][po:po + 64, col:col + NB],
                        lhsT=(w1a if po == 0 else w1b)[:],
                        rhs=xslot[:, xbase + c * NB:xbase + (c + 1) * NB],
                        start=True, stop=True,
                        tile_position=(0, po))
                last.then_inc(s_mm1, 1)
                if g >= MM2_LAG:
                    gp = g - MM2_LAG
                    jp, ggp = divmod(gp, NG)
                    if g == MM2_LAG:
                        tensor.wait_ge(s_w2p, 1)
                    tensor.wait_ge(s_hs, gp + 1)
                    if ggp == 0 and jp >= 2:
                        tensor.wait_ge(s_pan, jp - 1)
                    mm2_pair(gp)
            # tail: mm2s of the last MM2_LAG groups
            for gp in range(NGT - MM2_LAG, NGT):
                tensor.wait_ge(s_hs, gp + 1)
                jp, ggp = divmod(gp, NG)
                if ggp == 0 and jp >= 2:
                    tensor.wait_ge(s_pan, jp - 1)
                mm2_pair(gp)

        # ---------------- ACT: one [128,2NB] silu per group -------------
        @block.scalar
        def _(scalar):
            func = (mybir.ActivationFunctionType.Silu if use_silu
                    else mybir.ActivationFunctionType.Sigmoid)
            scalar.wait_ge(s_w, 80)
            for g in range(NGT):
                scalar.wait_ge(s_mm1, g + 1)
                # WAR: hs[g%NHS] reused -> mm2(g-NHS) must be done
                if g >= NHS:
                    scalar.wait_ge(s_mm2, 2 * (g - NHS + 1))
                nc.scalar.activation(
                    out=hs[g % NHS][:], in_=hp[g % 3][:],
                    func=func, bias=b1t[:], scale=1.0,
                ).then_inc(s_hs, 1)

        # ---------------- GPSIMD: w2p zero-fill ------------------------
        @block.gpsimd
        def _(gp):
            nc.gpsimd.memset(w2p[:], 0.0).then_inc(s_wz, 1)

        # ---------------- DVE: w2p fill, drains + incremental epilogue --
        @block.vector
        def _(vector):
            tick = [0]

            def step(ins):
                ins.then_inc(s_dve, 1)
                tick[0] += 1
                vector.wait_ge(s_dve, tick[0])

            # build w2p placement stationaries: block k has W2 lo at col
            # 130k (partitions 0:64) and W2 hi at col 130k+1 (64:128)
            vector.wait_ge(s_w, 80)
            vector.wait_ge(s_wz, 1)
            base_lo = w2p[0:64, 0:1]
            view_lo = bass.AP(tensor=base_lo.tensor, offset=base_lo.offset,
                              ap=[list(base_lo.ap[0]), [130, NPAIR]])
            base_hi = w2p[64:128, 1:2]
            view_hi = bass.AP(tensor=base_hi.tensor, offset=base_hi.offset,
                              ap=[list(base_hi.ap[0]), [130, NPAIR]])
            step(nc.vector.tensor_scalar_add(view_lo, view_lo, w2s[0:64, 0:1]))
            nc.vector.tensor_scalar_add(
                view_hi, view_hi, w2s[64:128, 0:1]).then_inc(s_w2p, 1)

            vector.wait_ge(s_aux, 32)
            step(nc.vector.tensor_tensor_scan(
                out=CH[:], data0=mAt, data1=pCHt,
                initial=0.0, op0=AOp.mult, op1=AOp.add))
            step(nc.vector.tensor_tensor_scan(
                out=IV[:], data0=mAt, data1=pIVt,
                initial=0.0, op0=AOp.mult, op1=AOp.add))
            step(nc.vector.tensor_scalar_add(IVp[:], IV[:], 1.0))

            for j in range(NBLK):
                lo, hi = j * NB, (j + 1) * NB
                # panel j complete after 2*NG*(j+1) mm2 incs
                vector.wait_ge(s_mm2, 2 * NG * (j + 1))
                nc.vector.tensor_scalar_add(
                    ao[:, lo:hi], panels[j % 2][:], b2t[:]
                ).then_inc(s_pan, 1)
                vector.wait_ge(s_pan, j + 1)
                # chained forward segmented scan for this block
                init = 0.0 if j == 0 else FL[:, lo - 1:lo]
                step(nc.vector.tensor_tensor_scan(
                    out=FL[:, lo:hi], data0=mAt[:, lo:hi], data1=ao[:, lo:hi],
                    initial=init, op0=AOp.mult, op1=AOp.add))
                # Q_j = (CH - FL)*IV + ao*(1+IV), accumulated into CH
                step(nc.vector.tensor_sub(CH[:, lo:hi], CH[:, lo:hi], FL[:, lo:hi]))
                step(nc.vector.tensor_mul(CH[:, lo:hi], CH[:, lo:hi], IV[:, lo:hi]))
                step(nc.vector.tensor_mul(RLr[:, lo:hi], ao[:, lo:hi], IVp[:, lo:hi]))
                step(nc.vector.tensor_add(CH[:, lo:hi], CH[:, lo:hi], RLr[:, lo:hi]))
            # tail: 2-chunk reverse scan; hi-half output DMAs while the
            # lo half is still being computed
            step(nc.vector.tensor_tensor_scan(
                out=RLr[:, 0:T2], data0=mBrt[:, 0:T2], data1=rev(ao[:], n=T2),
                initial=0.0, op0=AOp.mult, op1=AOp.add))
            step(nc.vector.tensor_mul(
                IV[:, T2:T], rev(RLr[:], n=T2, end=T2), IV[:, T2:T]))
            nc.vector.tensor_sub(
                obuf[:, T2:T], CH[:, T2:T], IV[:, T2:T]).then_inc(s_eh, 1)
            step(nc.vector.tensor_tensor_scan(
                out=RLr[:, T2:T], data0=mBrt[:, T2:T],
                data1=rev(ao[:], n=T2, end=T2),
                initial=RLr[:, T2 - 1:T2], op0=AOp.mult, op1=AOp.add))
            step(nc.vector.tensor_mul(IV[:, 0:T2], rev(RLr[:], n=T2), IV[:, 0:T2]))
            nc.vector.tensor_sub(
                obuf[:, 0:T2], CH[:, 0:T2], IV[:, 0:T2]).then_inc(s_el, 1)

    return nc


def build_nc(T, use_silu=True):
    key = (T, use_silu)
    if key in _NC_CACHE:
        return _NC_CACHE[key]
    nc = build_raw(T, use_silu=use_silu)
    _NC_CACHE[key] = nc
    return nc


def _pack(batch, charge, T):
    """Pack molecules into 1024 rows of capacity T. Returns per-atom slot
    positions and the host-side mask/value grids, or None if the
    molecules don't fit."""
    n = batch.shape[0]
    sizes = np.bincount(batch, minlength=N_MOL).astype(np.int64)
    nz = np.flatnonzero(sizes)           # non-empty molecules, in order
    szs = sizes[nz]
    nrows = NCORES * R

    # greedy sequential packing of molecules into rows
    row_of = np.empty(len(nz), np.int64)
    fstart = np.empty(len(nz), np.int64)
    r, f = 0, 0
    for i, sz in enumerate(szs):
        if f + sz > T:
            r += 1
            f = 0
        row_of[i] = r
        fstart[i] = f
        f += sz
    if r >= nrows:
        return None                      # doesn't fit at this T

    slot_start = row_of * T + fstart     # global slot of each molecule start
    # per-atom global slot: atoms of molecule i occupy slot_start[i] + 0..sz
    mol_atom_start = np.concatenate([[0], np.cumsum(szs)])[:-1]
    # batch is sorted, so atom a belongs to the idx-th non-empty molecule
    idx_of_atom = np.repeat(np.arange(len(nz)), szs)
    pos_of_atom = slot_start[idx_of_atom] + (np.arange(n) - mol_atom_start[idx_of_atom])

    # masks / placed values over all rows
    fill = np.zeros(nrows, np.int64)
    np.add.at(fill, row_of, szs)
    col = np.arange(T)
    mA = np.ones((nrows, T), np.float32)
    mA.reshape(-1)[slot_start] = 0.0
    mA[col[None, :] >= fill[:, None]] = 0.0
    slot_end = slot_start + szs - 1
    mBr = np.ones((nrows, T), np.float32)
    # reversed coords: slot (r, f) -> (r, T-1-f)
    mBr.reshape(-1)[(slot_end // T) * T + (T - 1 - (slot_end % T))] = 0.0
    # pad slots in reversed coords are cols < T - fill
    mBr[col[None, :] < (T - fill[:, None])] = 0.0

    pCH = np.zeros((nrows, T), np.float32)
    pCH.reshape(-1)[slot_start] = charge[nz]
    pIV = np.zeros((nrows, T), np.float32)
    pIV.reshape(-1)[slot_start] = (1.0 / szs).astype(np.float32)

    return pos_of_atom, mA, mBr, pCH, pIV


def kernel(x_scalar, batch, charge, W1, b1, W2, b2):
    x_scalar = np.asarray(x_scalar, dtype=np.float32)
    batch = np.asarray(batch, dtype=np.int32)
    charge = np.asarray(charge, dtype=np.float32)
    W1 = np.asarray(W1, dtype=np.float32)
    b1 = np.asarray(b1, dtype=np.float32)
    W2 = np.asarray(W2, dtype=np.float32)
    b2 = np.asarray(b2, dtype=np.float32)
    n = x_scalar.shape[0]

    # tolerate unsorted batch (reference data is sorted; this is insurance)
    order = None
    if np.any(np.diff(batch) < 0):
        order = np.argsort(batch, kind="stable")
        x_scalar = x_scalar[order]
        batch = batch[order]

    T = T_TIGHT
    packed = _pack(batch, charge, T)
    if packed is None:
        T = T_SAFE
        packed = _pack(batch, charge, T)
        assert packed is not None, "molecules do not fit even at T=2048"
    pos_of_atom, mA, mBr, pCH, pIV = packed
    NB = T // 4
    NPAIR = R // 2
    NBLK = 4
    S = R * T

    # padded, packed, transposed x per core (bf16)
    xpad = np.zeros((NCORES * S, D), NP_BF16)
    xpad[pos_of_atom] = x_scalar.astype(NP_BF16)
    xT_cores = []
    for c in range(NCORES):
        a = xpad[c * S:(c + 1) * S].reshape(NPAIR, 2, NBLK, NB, D)
        a = a.transpose(2, 0, 1, 3, 4).reshape(S, D)   # j-major stream order
        xT_cores.append(np.ascontiguousarray(a.T))
    del xpad

    W2s = np.concatenate([W2[:, 0], W2[:, 0]]).reshape(D, 1).astype(np.float32)
    W1 = W1.astype(NP_BF16)
    b1s = np.concatenate([b1, b1]).astype(np.float32)

    nc = build_nc(T, use_silu=True)
    in_maps = []
    for c in range(NCORES):
        sl = slice(c * R, (c + 1) * R)
        auxc = np.concatenate([mA[sl], mBr[sl], pCH[sl], pIV[sl]],
                              axis=1).astype(NP_BF16)
        in_maps.append({
            "xT": xT_cores[c], "W1": W1, "b1s": b1s, "b2": b2, "W2s": W2s,
            "aux": np.ascontiguousarray(auxc),
        })

    import os
    trace = bool(int(os.environ.get("ATOMIC_TRACE", "0")))
    res = run_bass_kernel_spmd(nc, in_maps, list(range(NCORES)), trace=trace)
    LAST_RUN_INFO["exec_time_ns"] = getattr(res, "exec_time_ns", None)
    LAST_RUN_INFO["profile_json"] = getattr(res, "profile_json", None)

    big = np.concatenate([res.results[c]["out"].reshape(-1).astype(np.float32)
                          for c in range(NCORES)])
    at = big[pos_of_atom]
    if order is not None:
        inv = np.empty_like(order)
        inv[order] = np.arange(n)
        at = at[inv]
    return at


# revision 9
# speedup vs baseline: 1.1979x; 1.0022x over previous
"""AtomicCharge Trainium2 kernel (nn_AtomicCharge_77781857730661).

Strategy
--------
Data-parallel over atoms across 8 NeuronCores. The host packs molecules
(contiguous runs of the sorted `batch` tensor) into 1024 partition-rows
(8 cores x 128 partitions) of capacity T slots (T=1984 when the data
packs that tight, else 2048), so every molecule lives contiguously
along the free dim of one partition. x is uploaded pre-transposed in
bf16 (halves HBM traffic vs fp32), in the j-major order the device
pipeline streams it. aux masks travel bf16; output returns bf16.

Per core (raw bass, explicit semaphores; waits are standalone):
  PE:  per group (4 chunks x NB atoms): 4x mm1 (W1^T x, bf16) col-tiled
       2x on the PE array -- chunks 0/2 via tile (0,0) into hp[0:64],
       chunks 1/3 via tile (0,64) into hp[64:128]; consecutive
       tile-0/tile-64 matmuls stream concurrently into one [128,2NB]
       PSUM tile per group. Then 2x K=128 mm2 with per-pair placement
       stationaries accumulate atom_out into the packed [128,NB] panel
       PSUM for block j (panels double-buffered; mm2 lagged 6 groups).
       The mm2 placement stationary w2p is BUILT ON DEVICE (gpsimd
       memset + 2 strided DVE broadcast-adds) instead of DMAing 2.1MB.
  ACT: ONE [128,2NB] Silu per group (bias b1 fused, bf16 out).
  DVE: per-j panel drains (+b2), chained per-block forward segmented
       scans and Q = (CH-FL)*IV + ao*(1+IV) precompute; the tail is a
       2-chunk reverse scan with the hi-half elementwise finish on
       GPSIMD in parallel, and the output DMAs per half:
       out = Q - rev(RL)*IV.

Pipelining: x slabs 4 groups each (~16KB/partition DMA lines), 4-deep;
hp PSUM triple-buffered; hs 8-deep.
The compile enables walrus redundant-LDWEIGHTS elimination.
HW: ~200 us/core on trn2 (8 cores); rel err ~4e-3 vs fp32 ref (bf16).
"""
import sys

sys.path.insert(0, "/opt/trn_rl_repo")

import numpy as np
import ml_dtypes

import concourse.bass as bass
from concourse import mybir
from concourse.bass_utils import run_bass_kernel_spmd
import concourse.bass_utils as _bu

# Enable walrus's redundant-LDWEIGHTS elimination (off by default in this
# stack); our mm1s reuse the same stationaries within a group.
if not getattr(_bu, "_ldwopt_patched", False):
    _orig_run_command = _bu.run_command

    def _run_command_ldwopt(argv, **kw):
        argv = [a.replace("--enable-ldw-opt=false", "--enable-ldw-opt=true")
                for a in argv]
        return _orig_run_command(argv, **kw)

    _bu.run_command = _run_command_ldwopt
    _bu._ldwopt_patched = True

F32 = mybir.dt.float32
BF16 = mybir.dt.bfloat16
NP_BF16 = ml_dtypes.bfloat16

# problem constants (hardcoded per spec)
N_ATOMS = 2_000_000
N_MOL = 50_000
D = 128      # node feature dim = SBUF partitions
H = 64       # hidden dim
NCORES = 8
R = 128      # atom-layout rows per core (partitions)
T_TIGHT = 1984
T_SAFE = 2048

_NC_CACHE = {}
LAST_RUN_INFO = {}


def build_raw(T, use_silu=True):
    """j-outer pipeline: groups of 2 pairs; col-tiled mm1 into one
    [128,2NB] PSUM tile; one [128,2NB] silu; K=128 mm2; panels
    double-buffered across j; incremental DVE epilogue."""
    NB = T // 4
    NPAIR = R // 2
    NBLK = 4
    NG = NPAIR // 2             # groups per block; group = 2 pairs = 4*NB atoms
    S = R * T
    CW = 512                    # chunk stride inside hp/hs (bank-aligned)
    WW = 2 * CW                 # hs width per group (pad cols beyond NB unused)
    GW = 4 * NB                 # xT columns per group (tight, no padding)
    XPG = 4                     # groups per x slab
    NXP = 5                     # x slab buffers
    NHS = 10                    # hs buffers
    MM2_LAG = 6                 # mm2 trails mm1 by this many groups
    T2 = T // 2
    AOp = mybir.AluOpType

    nc = bass.Bass()
    # xT is laid out j-major on the host: block j, then pair k, then
    # (row 2k | row 2k+1) x NB columns
    xT = nc.declare_dram_parameter("xT", [D, S], BF16, isOutput=False)
    W1 = nc.declare_dram_parameter("W1", [D, H], BF16, isOutput=False)
    b1s = nc.declare_dram_parameter("b1s", [D], F32, isOutput=False)
    b2 = nc.declare_dram_parameter("b2", [1], F32, isOutput=False)
    W2s = nc.declare_dram_parameter("W2s", [D, 1], F32, isOutput=False)
    aux = nc.declare_dram_parameter("aux", [R, 4 * T], BF16, isOutput=False)
    out = nc.declare_dram_parameter("out", [R, T], BF16, isOutput=True)

    from contextlib import ExitStack
    with ExitStack() as ctx:
        def sbuf(shape, dtype, name):
            return ctx.enter_context(nc.sbuf_tensor(name, shape, dtype))

        def psum(shape, name):
            return ctx.enter_context(nc.psum_tensor(name, shape, F32))

        w1a = sbuf([D, H], BF16, "w1a")
        w1b = sbuf([D, H], BF16, "w1b")
        b1t = sbuf([D, 1], F32, "b1t")
        b2t = sbuf([D, 1], F32, "b2t")
        w2s = sbuf([D, 1], F32, "w2s")
        w2p = sbuf([D, NPAIR * D], BF16, "w2p")
        auxt = sbuf([R, 4 * T], BF16, "auxt")
        xp = [sbuf([D, XPG * GW], BF16, f"xp{s}") for s in range(NXP)]
        hs = [sbuf([D, WW], BF16, f"hs{s}") for s in range(NHS)]
        ao = sbuf([R, T], F32, "ao")
        FL = sbuf([R, T], F32, "FL")
        RLr = sbuf([R, T], F32, "RLr")
        CH = sbuf([R, T], F32, "CH")
        IV = sbuf([R, T], F32, "IV")
        aoiv = sbuf([R, T], F32, "aoiv")
        obuf = sbuf([R, T], BF16, "obuf")

        hp = [psum([D, WW], f"hp{s}") for s in range(3)]     # 2 banks each
        panels = [psum([R, NB], f"panel{s}") for s in range(2)]

        s_w = ctx.enter_context(nc.semaphore("s_w"))
        s_wz = ctx.enter_context(nc.semaphore("s_wz"))
        s_w2p = ctx.enter_context(nc.semaphore("s_w2p"))
        s_aux = ctx.enter_context(nc.semaphore("s_aux"))
        s_x = [ctx.enter_context(nc.semaphore(f"s_x{i}")) for i in range(NXP)]
        s_mm1 = ctx.enter_context(nc.semaphore("s_mm1"))
        s_hs = ctx.enter_context(nc.semaphore("s_hs"))
        s_mm2 = ctx.enter_context(nc.semaphore("s_mm2"))
        s_pan = ctx.enter_context(nc.semaphore("s_pan"))
        s_dve = ctx.enter_context(nc.semaphore("s_dve"))
        s_sA = ctx.enter_context(nc.semaphore("s_sA"))
        s_gp = ctx.enter_context(nc.semaphore("s_gp"))
        s_eh = ctx.enter_context(nc.semaphore("s_eh"))
        s_el = ctx.enter_context(nc.semaphore("s_el"))
        s_out = ctx.enter_context(nc.semaphore("s_out"))
        block = ctx.enter_context(nc.Block())

        mAt = auxt[:, 0 * T:1 * T]
        mBrt = auxt[:, 1 * T:2 * T]
        pCHt = auxt[:, 2 * T:3 * T]
        pIVt = auxt[:, 3 * T:4 * T]

        def rev(ap, n=None, end=None):
            """Reverse view over the free dim: elements end-1, end-2, ...
            end-n (defaults: end=T, n=T)."""
            n = T if n is None else n
            end = T if end is None else end
            return bass.AP(tensor=ap.tensor, offset=ap.offset + (end - 1),
                           ap=[list(ap.ap[0]), [-1, n]])

        NGT = NBLK * NG          # total groups = 128
        NSLAB = NGT // XPG       # 32 x slabs
        SLW = XPG * GW           # slab width in xT columns

        # slab plan: 4-group slabs, then single-group slabs at the end so
        # the final groups pipeline through PE/ACT instead of arriving as
        # one 4-group burst
        SLABS = [(4 * i, 4) for i in range(30)] + [(120 + i, 1) for i in range(8)]
        slab_of_group = {}
        for i, (st, n) in enumerate(SLABS):
            for k in range(n):
                slab_of_group[st + k] = i

        # ---------------- SP: all DMA traffic ----------------
        @block.sync
        def _(sync):
            def xdma(i):
                st, n = SLABS[i]
                sync.dma_start(out=xp[i % NXP][:, 0:n * GW],
                               in_=xT[:, st * GW:(st + n) * GW]
                               ).then_inc(s_x[i % NXP], 16)

            xdma(0)
            sync.dma_start(out=w1a[:], in_=W1[:]).then_inc(s_w, 16)
            sync.dma_start(out=w1b[:], in_=W1[:]).then_inc(s_w, 16)
            sync.dma_start(out=b1t[:], in_=b1s[:, None]).then_inc(s_w, 16)
            b2bc = bass.AP(tensor=b2.ap().tensor, offset=0, ap=[[0, D], [1, 1]])
            sync.dma_start(out=b2t[:], in_=b2bc).then_inc(s_w, 16)
            sync.dma_start(out=w2s[:], in_=W2s[:]).then_inc(s_w, 16)
            for i in range(1, NXP):
                xdma(i)
            for i in range(NXP, len(SLABS)):
                # slab slot free once its previous tenant's mm1s consumed
                pst, pn = SLABS[i - NXP]
                sync.wait_ge(s_mm1, pst + pn)
                xdma(i)
                if i == NXP:
                    sync.dma_start(out=auxt[:, 0:2 * T],
                                   in_=aux[:, 0:2 * T]).then_inc(s_aux, 16)
                elif i == NXP + 1:
                    sync.dma_start(out=auxt[:, 2 * T:4 * T],
                                   in_=aux[:, 2 * T:4 * T]).then_inc(s_aux, 16)
            sync.wait_ge(s_eh, 1)
            sync.dma_start(out=out[:, T2:T], in_=obuf[:, T2:T]).then_inc(s_out, 16)
            sync.wait_ge(s_el, 1)
            sync.dma_start(out=out[:, 0:T2], in_=obuf[:, 0:T2]).then_inc(s_out, 16)
            sync.wait_ge(s_out, 32)

        # ---------------- PE ----------------
        @block.tensor
        def _(tensor):
            tensor.wait_ge(s_w, 80)

            def mm2_pair(gp):
                jp, ggp = divmod(gp, NG)
                for c in range(2):
                    kp = 2 * ggp + c
                    nc.tensor.matmul(
                        out=panels[jp % 2][:],
                        lhsT=w2p[:, kp * D:(kp + 1) * D],
                        rhs=hs[gp % NHS][:, c * CW:c * CW + NB],
                        start=(ggp == 0 and c == 0),
                        stop=(ggp == NG - 1 and c == 1)).then_inc(s_mm2, 1)

            for g in range(NGT):
                i = slab_of_group[g]
                st, n = SLABS[i]
                if g == st:
                    tensor.wait_ge(s_x[i % NXP], 16 * (i // NXP + 1))
                # WAR: hp[g%3] reused -> silu(g-3) must be done
                if g >= 3:
                    tensor.wait_ge(s_hs, g - 2)
                xbase = (g - st) * GW
                xslot = xp[i % NXP]
                last = None
                for c in range(4):
                    po = 64 * (c & 1)          # chunks 0,2 -> rows 0:64; 1,3 -> 64:128
                    col = CW * (c >> 1)        # chunks 0,1 -> cols 0:NB; 2,3 -> CW:
                    last = nc.tensor.matmul(
                        out=hp[g % 3][po:po + 64, col:col + NB],
                        lhsT=(w1a if po == 0 else w1b)[:],
                        rhs=xslot[:, xbase + c * NB:xbase + (c + 1) * NB],
                        start=True, stop=True,
                        tile_position=(0, po))
                last.then_inc(s_mm1, 1)
                if g >= MM2_LAG:
                    gp = g - MM2_LAG
                    jp, ggp = divmod(gp, NG)
                    if g == MM2_LAG:
                        tensor.wait_ge(s_w2p, 1)
                    tensor.wait_ge(s_hs, gp + 1)
                    if ggp == 0 and jp >= 2:
                        tensor.wait_ge(s_pan, jp - 1)
                    mm2_pair(gp)
            # tail: mm2s of the last MM2_LAG groups
            for gp in range(NGT - MM2_LAG, NGT):
                tensor.wait_ge(s_hs, gp + 1)
                jp, ggp = divmod(gp, NG)
                if ggp == 0 and jp >= 2:
                    tensor.wait_ge(s_pan, jp - 1)
                mm2_pair(gp)

        # ---------------- ACT: one [128,2NB] silu per group -------------
        @block.scalar
        def _(scalar):
            func = (mybir.ActivationFunctionType.Silu if use_silu
                    else mybir.ActivationFunctionType.Sigmoid)
            scalar.wait_ge(s_w, 80)
            for g in range(NGT):
                scalar.wait_ge(s_mm1, g + 1)
                # WAR: hs[g%NHS] reused -> mm2(g-NHS) must be done
                if g >= NHS:
                    scalar.wait_ge(s_mm2, 2 * (g - NHS + 1))
                nc.scalar.activation(
                    out=hs[g % NHS][:], in_=hp[g % 3][:],
                    func=func, bias=b1t[:], scale=1.0,
                ).then_inc(s_hs, 1)

        # ---------------- GPSIMD: w2p zero-fill ------------------------
        @block.gpsimd
        def _(gp):
            nc.gpsimd.memset(w2p[:], 0.0).then_inc(s_wz, 1)

        # ---------------- DVE: w2p fill, drains + incremental epilogue --
        @block.vector
        def _(vector):
            tick = [0]

            def step(ins):
                ins.then_inc(s_dve, 1)
                tick[0] += 1
                vector.wait_ge(s_dve, tick[0])

            # build w2p placement stationaries: block k has W2 lo at col
            # 130k (partitions 0:64) and W2 hi at col 130k+1 (64:128)
            vector.wait_ge(s_w, 80)
            vector.wait_ge(s_wz, 1)
            base_lo = w2p[0:64, 0:1]
            view_lo = bass.AP(tensor=base_lo.tensor, offset=base_lo.offset,
                              ap=[list(base_lo.ap[0]), [130, NPAIR]])
            base_hi = w2p[64:128, 1:2]
            view_hi = bass.AP(tensor=base_hi.tensor, offset=base_hi.offset,
                              ap=[list(base_hi.ap[0]), [130, NPAIR]])
            step(nc.vector.tensor_scalar_add(view_lo, view_lo, w2s[0:64, 0:1]))
            nc.vector.tensor_scalar_add(
                view_hi, view_hi, w2s[64:128, 0:1]).then_inc(s_w2p, 1)

            vector.wait_ge(s_aux, 32)
            # CH scan expands charge/cnt (host supplies charge/cnt at
            # segment starts); IV expands 1/cnt
            step(nc.vector.tensor_tensor_scan(
                out=CH[:], data0=mAt, data1=pCHt,
                initial=0.0, op0=AOp.mult, op1=AOp.add))
            step(nc.vector.tensor_tensor_scan(
                out=IV[:], data0=mAt, data1=pIVt,
                initial=0.0, op0=AOp.mult, op1=AOp.add))

            for j in range(NBLK):
                lo, hi = j * NB, (j + 1) * NB
                # panel j complete after 2*NG*(j+1) mm2 incs
                vector.wait_ge(s_mm2, 2 * NG * (j + 1))
                nc.vector.tensor_scalar_add(
                    ao[:, lo:hi], panels[j % 2][:], b2t[:]
                ).then_inc(s_pan, 1)
                vector.wait_ge(s_pan, j + 1)
                # aoiv = ao/cnt; FL = chained forward segmented scan of aoiv;
                # Q_j = ao + CH - FL + aoiv accumulated into CH
                step(nc.vector.tensor_mul(
                    aoiv[:, lo:hi], ao[:, lo:hi], IV[:, lo:hi]))
                init = 0.0 if j == 0 else FL[:, lo - 1:lo]
                step(nc.vector.tensor_tensor_scan(
                    out=FL[:, lo:hi], data0=mAt[:, lo:hi], data1=aoiv[:, lo:hi],
                    initial=init, op0=AOp.mult, op1=AOp.add))
                step(nc.vector.tensor_sub(CH[:, lo:hi], CH[:, lo:hi], FL[:, lo:hi]))
                step(nc.vector.tensor_add(CH[:, lo:hi], CH[:, lo:hi], ao[:, lo:hi]))
                step(nc.vector.tensor_add(CH[:, lo:hi], CH[:, lo:hi], aoiv[:, lo:hi]))
            # tail: 2-chunk reverse scan of aoiv; out = Q - rev(scan);
            # hi-half output DMAs while the lo half is still being computed
            step(nc.vector.tensor_tensor_scan(
                out=RLr[:, 0:T2], data0=mBrt[:, 0:T2], data1=rev(aoiv[:], n=T2),
                initial=0.0, op0=AOp.mult, op1=AOp.add))
            nc.vector.tensor_sub(
                obuf[:, T2:T], CH[:, T2:T], rev(RLr[:], n=T2, end=T2)
            ).then_inc(s_eh, 1)
            step(nc.vector.tensor_tensor_scan(
                out=RLr[:, T2:T], data0=mBrt[:, T2:T],
                data1=rev(aoiv[:], n=T2, end=T2),
                initial=RLr[:, T2 - 1:T2], op0=AOp.mult, op1=AOp.add))
            nc.vector.tensor_sub(
                obuf[:, 0:T2], CH[:, 0:T2], rev(RLr[:], n=T2)
            ).then_inc(s_el, 1)

    return nc


def build_nc(T, use_silu=True):
    key = (T, use_silu)
    if key in _NC_CACHE:
        return _NC_CACHE[key]
    nc = build_raw(T, use_silu=use_silu)
    _NC_CACHE[key] = nc
    return nc


def _pack(batch, charge, T):
    """Pack molecules into 1024 rows of capacity T. Returns per-atom slot
    positions and the host-side mask/value grids, or None if the
    molecules don't fit."""
    n = batch.shape[0]
    sizes = np.bincount(batch, minlength=N_MOL).astype(np.int64)
    nz = np.flatnonzero(sizes)           # non-empty molecules, in order
    szs = sizes[nz]
    nrows = NCORES * R

    # greedy sequential packing of molecules into rows
    row_of = np.empty(len(nz), np.int64)
    fstart = np.empty(len(nz), np.int64)
    r, f = 0, 0
    for i, sz in enumerate(szs):
        if f + sz > T:
            r += 1
            f = 0
        row_of[i] = r
        fstart[i] = f
        f += sz
    if r >= nrows:
        return None                      # doesn't fit at this T

    slot_start = row_of * T + fstart     # global slot of each molecule start
    # per-atom global slot: atoms of molecule i occupy slot_start[i] + 0..sz
    mol_atom_start = np.concatenate([[0], np.cumsum(szs)])[:-1]
    # batch is sorted, so atom a belongs to the idx-th non-empty molecule
    idx_of_atom = np.repeat(np.arange(len(nz)), szs)
    pos_of_atom = slot_start[idx_of_atom] + (np.arange(n) - mol_atom_start[idx_of_atom])

    # masks / placed values over all rows
    fill = np.zeros(nrows, np.int64)
    np.add.at(fill, row_of, szs)
    col = np.arange(T)
    mA = np.ones((nrows, T), np.float32)
    mA.reshape(-1)[slot_start] = 0.0
    mA[col[None, :] >= fill[:, None]] = 0.0
    slot_end = slot_start + szs - 1
    mBr = np.ones((nrows, T), np.float32)
    # reversed coords: slot (r, f) -> (r, T-1-f)
    mBr.reshape(-1)[(slot_end // T) * T + (T - 1 - (slot_end % T))] = 0.0
    # pad slots in reversed coords are cols < T - fill
    mBr[col[None, :] < (T - fill[:, None])] = 0.0

    pCH = np.zeros((nrows, T), np.float32)
    pCH.reshape(-1)[slot_start] = (charge[nz] / szs).astype(np.float32)
    pIV = np.zeros((nrows, T), np.float32)
    pIV.reshape(-1)[slot_start] = (1.0 / szs).astype(np.float32)

    return pos_of_atom, mA, mBr, pCH, pIV


def kernel(x_scalar, batch, charge, W1, b1, W2, b2):
    x_scalar = np.asarray(x_scalar, dtype=np.float32)
    batch = np.asarray(batch, dtype=np.int32)
    charge = np.asarray(charge, dtype=np.float32)
    W1 = np.asarray(W1, dtype=np.float32)
    b1 = np.asarray(b1, dtype=np.float32)
    W2 = np.asarray(W2, dtype=np.float32)
    b2 = np.asarray(b2, dtype=np.float32)
    n = x_scalar.shape[0]

    # tolerate unsorted batch (reference data is sorted; this is insurance)
    order = None
    if np.any(np.diff(batch) < 0):
        order = np.argsort(batch, kind="stable")
        x_scalar = x_scalar[order]
        batch = batch[order]

    T = T_TIGHT
    packed = _pack(batch, charge, T)
    if packed is None:
        T = T_SAFE
        packed = _pack(batch, charge, T)
        assert packed is not None, "molecules do not fit even at T=2048"
    pos_of_atom, mA, mBr, pCH, pIV = packed
    NB = T // 4
    NPAIR = R // 2
    NBLK = 4
    S = R * T

    # padded, packed, transposed x per core (bf16)
    xpad = np.zeros((NCORES * S, D), NP_BF16)
    xpad[pos_of_atom] = x_scalar.astype(NP_BF16)
    xT_cores = []
    for c in range(NCORES):
        a = xpad[c * S:(c + 1) * S].reshape(NPAIR, 2, NBLK, NB, D)
        a = a.transpose(2, 0, 1, 3, 4).reshape(S, D)   # j-major stream order
        xT_cores.append(np.ascontiguousarray(a.T))
    del xpad

    W2s = np.concatenate([W2[:, 0], W2[:, 0]]).reshape(D, 1).astype(np.float32)
    W1 = W1.astype(NP_BF16)
    b1s = np.concatenate([b1, b1]).astype(np.float32)

    nc = build_nc(T, use_silu=True)
    in_maps = []
    for c in range(NCORES):
        sl = slice(c * R, (c + 1) * R)
        auxc = np.concatenate([mA[sl], mBr[sl], pCH[sl], pIV[sl]],
                              axis=1).astype(NP_BF16)
        in_maps.append({
            "xT": xT_cores[c], "W1": W1, "b1s": b1s, "b2": b2, "W2s": W2s,
            "aux": np.ascontiguousarray(auxc),
        })

    import os
    trace = bool(int(os.environ.get("ATOMIC_TRACE", "0")))
    res = run_bass_kernel_spmd(nc, in_maps, list(range(NCORES)), trace=trace)
    LAST_RUN_INFO["exec_time_ns"] = getattr(res, "exec_time_ns", None)
    LAST_RUN_INFO["profile_json"] = getattr(res, "profile_json", None)

    big = np.concatenate([res.results[c]["out"].reshape(-1).astype(np.float32)
                          for c in range(NCORES)])
    at = big[pos_of_atom]
    if order is not None:
        inv = np.empty_like(order)
        inv[order] = np.arange(n)
        at = at[inv]
    return at
